# revision 23
# baseline (speedup 1.0000x reference)
"""Distributed attention block for Trainium2 (8 NeuronCores, SPMD).

Problem: B=2, S=2048, D=512, H=8 (head_dim = D = 512).
  qkv = einsum('bsd,dhf->bshf', x, w_qkv) + b_qkv     f = 3*D
  q, k, v = split(qkv); weights = softmax(q @ k^T / sqrt(D))
  out = einsum('bqhd,hdo->bqo', weights @ v, w_out) + b_out

Sharding: head-parallel (one head per core); per-half-chunk bf16
ReduceScatters sum the 8 partial output projections.

The projection algebra is folded down to three matmul stages per head:
  scores = q k^T = x (Wq Wk^T) x^T   -> one z = x@Wqk projection (bf16)
                                        instead of separate Q and K
  V W_out = x (Wv W_out) + bv W_out  -> V projection eliminated; Wvo = Wv@Wout
                                        precomputed host-side (bf16)
Bias exactness: softmax is invariant to per-query score offsets, so the
(x Wq)bk^T and bq bk^T terms cancel; the per-key term bq.(x Wk) is folded
into the Exp activation's per-partition bias (aux input eb, host-computed);
bv@W_out and b_out are added host-side.

The scores matmul runs in fp8 e4m3 with MatmulPerfMode.DoubleRow (256-row
contraction pairs, 2x bf16 throughput): x^T is quantized host-side into a
c-pair layout [p, c2, i, t]; z^T is quantized out of PSUM by the scalar
engine (Copy, x16 prescale folded back out in the Exp scale). VW and PV
stay bf16 — fp8 on the value path costs too much precision.

Row-sums: DVE pair+quad partial sums over the 16 E^T tiles as the exps
complete, then 4 accumulated all-ones matmuls (every PSUM row then holds
the same sums = free partition-broadcast reciprocal). Normalization is
fused into the bf16 Y^T eviction multiply. Each 512-row y chunk
reduce-scatters as two 256-row halves, issued as soon as their ob-pair is
evicted, into one contiguous rs_all buffer whose byte layout equals the
output shard's — so finishing is just two linear per-batch DMAs (emitted
after zvw1/attn1 so scheduler hoisting can't head-of-line-block anything).
"""
import sys

for _p in ("/opt/trn_rl_repo",):
    if _p not in sys.path:
        sys.path.append(_p)

import numpy as np
import ml_dtypes

import concourse.bass as bass
import concourse.bacc as bacc
import concourse.mybir as mybir
import concourse.tile as tile
from concourse.bass import ts
from concourse.bass_utils import run_bass_kernel_spmd

BF16 = mybir.dt.bfloat16
F32 = mybir.dt.float32
F8 = mybir.dt.float8e4

B, S, D, H = 2, 2048, 512, 8
T = B * S                  # 4096 tokens
P = 128                    # partitions
NC = 8                     # cores
DC = D // P                # 4 contraction chunks of 128
FB = 512                   # moving free-dim per matmul
OUT_ROWS = D // NC         # 64 output-feature rows per core after RS
RS_HALF = D // 2           # 256-row reduce-scatter granule
SCALE = float(D) ** -0.5
Z_SCALE = 16.0             # fp8 prescale for z (values ~N(0, 0.2))
DR = mybir.MatmulPerfMode.DoubleRow
COPY = mybir.ActivationFunctionType.Copy

_CACHED = {}


def _build(s=S, debug=False):
    t_all = B * s
    nkb_all = t_all // P
    nc = bacc.Bacc(None, target_bir_lowering=False, debug=debug, num_devices=NC)

    xt_ext = nc.declare_dram_parameter("xt", [D, t_all], BF16, isOutput=False)
    x8_ext = nc.declare_dram_parameter("x8", [P, 4 * t_all], F8, isOutput=False)
    wqk_ext = nc.declare_dram_parameter("wqk", [D, D], BF16, isOutput=False)
    wvo_ext = nc.declare_dram_parameter("wvo", [D, D], BF16, isOutput=False)
    eb_ext = nc.declare_dram_parameter("eb", [P, nkb_all], F32, isOutput=False)
    # chunk-major output: [token-chunk, 64 shard rows, 512 tokens]
    out_ext = nc.declare_dram_parameter(
        "out", [t_all // FB, OUT_ROWS, FB], BF16, isOutput=True)

    with tile.TileContext(nc) as tc:
        with (
            tc.tile_pool(name="consts", bufs=1) as consts,
            tc.tile_pool(name="zvw_sb", bufs=1) as zvw_sb,
            tc.tile_pool(name="et_sb", bufs=2) as et_pool,
            tc.tile_pool(name="small", bufs=2) as small,
            tc.tile_pool(name="epair_sb", bufs=2) as epair_pool,
            tc.tile_pool(name="ysb", bufs=3) as ysb_pool,
            tc.tile_pool(name="ps_mm", bufs=5, space="PSUM") as ps_mm,
            tc.tile_pool(name="ps_sum", bufs=1, space="PSUM") as ps_sum,
            tc.tile_pool(name="ps_y", bufs=2, space="PSUM") as ps_y,
            tc.tile_pool(name="dram", bufs=1, space="DRAM") as dram,
        ):
            # ---- resident inputs, critical-path-first DMA order ----------------
            # z-proj consumes x^T (bf16) token-chunk by token-chunk; x8 (fp8
            # c-pair layout, for the scores stationary side) is needed later.
            xt_sb = consts.tile([P, DC, t_all], BF16)
            x8_sb = consts.tile([P, 2, 2, t_all], F8)
            wqk_sb = consts.tile([P, DC, D], BF16)
            wvo_sb = consts.tile([P, DC, D], BF16)
            eb_sb = consts.tile([P, nkb_all], F32)
            # first z-proj tile (t0, f0) needs only wqk's f0 columns + xt t0:
            # issue those ~640KB first so the first matmul fires at ~3us
            for c in range(DC):
                nc.sync.dma_start(wqk_sb[:, c, ts(0, P)],
                                  wqk_ext[ts(c, P), ts(0, P)])
                nc.scalar.dma_start(xt_sb[:, c, ts(0, FB)],
                                    xt_ext[ts(c, P), ts(0, FB)])
            for c in range(DC):
                nc.sync.dma_start(wqk_sb[:, c, P:D], wqk_ext[ts(c, P), P:D])
            nc.scalar.dma_start(eb_sb[:], eb_ext[:])
            for c in range(DC):
                nc.sync.dma_start(xt_sb[:, c, ts(1, FB)],
                                  xt_ext[ts(c, P), ts(1, FB)])
                nc.scalar.dma_start(xt_sb[:, c, ts(2, FB)],
                                    xt_ext[ts(c, P), ts(2, FB)])
            # gpsimd stream in batch-0-first consumption order: xt t3 (b0),
            # x8 b0 planes, then all of batch 1
            nhc = t_all // (2 * FB)
            for c in range(DC):
                nc.gpsimd.dma_start(xt_sb[:, c, ts(3, FB)],
                                    xt_ext[ts(c, P), ts(3, FB)])
            for c in range(DC):
                nc.sync.dma_start(wvo_sb[:, c, :], wvo_ext[ts(c, P), :])
            for half in range(2):
                if half == 1:
                    for t in range(nhc, 2 * nhc):
                        for c in range(DC):
                            nc.gpsimd.dma_start(xt_sb[:, c, ts(t, FB)],
                                                xt_ext[ts(c, P), ts(t, FB)])
                for t in range(half * nhc, (half + 1) * nhc):
                    for c2 in range(2):
                        for i in range(2):
                            o = (c2 * 2 + i) * t_all + t * FB
                            nc.gpsimd.dma_start(x8_sb[:, c2, i, ts(t, FB)],
                                                x8_ext[:, o: o + FB])
            ones_sb = consts.tile([P, P], BF16)
            nc.vector.memset(ones_sb[:], 1.0)

            # ---- per-batch working tiles (shared slots across batches) ---------
            # z^T fp8 c-pair layout [p, c2, i, t], chunk c = 2*c2 + i (x16)
            zt_sb = zvw_sb.tile([P, 2, 2, s], F8, tag="zt")
            vw_sb = zvw_sb.tile([P, s // P, D], BF16, tag="vw")

            y_ch = [[dram.tile([D, FB], BF16, name=f"y_ch{b}_{t}")
                     for t in range(s // FB)] for b in range(B)]
            # one contiguous RS landing zone, byte-layout-identical to out_ext
            rs_all = dram.tile([t_all // FB, OUT_ROWS, FB], BF16,
                               name="rs_all")

            def zvw_phase(b):
                t0 = b * s
                # z^T: psum [f=128, t=512] = sum_c Wqk-chunk.T @ x^T (bf16),
                # then the scalar engine quantizes x16 into fp8 (t outer so
                # each x^T token-chunk is consumed as soon as it lands).
                for t in range(s // FB):
                    for f in range(DC):
                        ps = ps_mm.tile([P, FB], F32, tag="ps")
                        for c in range(DC):
                            nc.tensor.matmul(
                                ps[:], wqk_sb[:, c, ts(f, P)],
                                xt_sb[:, c, t0 + t * FB: t0 + (t + 1) * FB],
                                start=(c == 0), stop=(c == DC - 1),
                            )
                        nc.scalar.activation(
                            zt_sb[:, f // 2, f % 2, ts(t, FB)], ps[:],
                            COPY, scale=Z_SCALE)
                # VW = x @ Wvo: psum [k=128, o=512] = x^T-chunk.T @ Wvo
                for kb in range(s // P):
                    ps = ps_mm.tile([P, D], F32, tag="ps")
                    for c in range(DC):
                        nc.tensor.matmul(
                            ps[:], xt_sb[:, c, t0 + kb * P: t0 + (kb + 1) * P],
                            wvo_sb[:, c, :],
                            start=(c == 0), stop=(c == DC - 1),
                        )
                    nc.vector.tensor_copy(vw_sb[:, kb, :], ps[:])

            def attn_phase(b):
                nkb = s // P
                t0 = b * s
                nqb = s // FB
                ets, eps = {}, {}

                def score_chunk(qb, kb_lo, kb_hi):
                    # scores+exp for kb in [kb_lo, kb_hi); <=4 tiles per call
                    # keeps the psum pool ahead of the exp drain
                    if kb_lo == 0:
                        ets[qb] = et_pool.tile([P, nkb, FB], BF16, tag="et",
                                               name=f"et_{b}_{qb}")
                        eps[qb] = epair_pool.tile([P, nkb // 4, 3, FB], BF16,
                                                  tag="epair",
                                                  name=f"epair_{b}_{qb}")
                    et_sb, epair = ets[qb], eps[qb]
                    for kb in range(kb_lo, kb_hi):
                        ps = ps_mm.tile([P, FB], F32, tag="ps")
                        # psum [k=128, q=512] = x8-pair.T @ z8-pair = 16*scores^T
                        for c2 in range(2):
                            nc.tensor.matmul(
                                ps[:], x8_sb[:, c2, :, t0 + kb * P: t0 + (kb + 1) * P],
                                zt_sb[:, c2, :, ts(qb, FB)],
                                start=(c2 == 0), stop=(c2 == 1),
                                perf_mode=DR,
                            )
                        # exp(scale*s + per-key bias) straight out of PSUM
                        nc.scalar.activation(
                            et_sb[:, kb, :], ps[:],
                            mybir.ActivationFunctionType.Exp,
                            scale=SCALE / Z_SCALE,
                            bias=eb_sb[:, b * nkb + kb: b * nkb + kb + 1],
                        )
                        if kb % 2 == 1:
                            nc.vector.tensor_add(
                                epair[:, kb // 4, kb // 2 % 2, :],
                                et_sb[:, kb - 1, :], et_sb[:, kb, :])
                        if kb % 4 == 3:
                            nc.vector.tensor_add(
                                epair[:, kb // 4, 2, :],
                                epair[:, kb // 4, 0, :], epair[:, kb // 4, 1, :])

                # software pipeline: next qb's score chunks are interleaved
                # into this qb's PV loop so the tensor queue never waits on
                # the exp drain or on a full rowsum tree
                score_chunk(0, 0, nkb)
                for qb in range(nqb):
                    et_sb = ets.pop(qb)
                    epair = eps.pop(qb)
                    # cross-partition rowsum via accumulated all-ones matmuls
                    ps_s = ps_sum.tile([P, FB], F32, tag="ps_sum")
                    for j in range(nkb // 4):
                        nc.tensor.matmul(ps_s[:], ones_sb[:], epair[:, j, 2, :],
                                         start=(j == 0), stop=(j == nkb // 4 - 1))
                    brecip = small.tile([P, FB], BF16, tag="brecip")
                    with nc.allow_low_precision(
                            reason="bf16 1/rowsum: 2x DVE rate; emulated "
                                   "rel-err delta 5e-5"):
                        nc.vector.reciprocal(brecip[:], ps_s[:])
                    # fused PV+output projection:
                    # psum [o=128, q=512] = VW-block.T @ E^T, normalize on evict.
                    # Reduce-scatter each 256-row half as soon as it's written.
                    cb = b * nqb + qb
                    for ob in range(DC):
                        if qb + 1 < nqb:
                            score_chunk(qb + 1, 4 * ob, 4 * ob + 4)
                        ps = ps_y.tile([P, FB], F32, tag="ps_y")
                        for kb in range(nkb):
                            nc.tensor.matmul(
                                ps[:], vw_sb[:, kb, ts(ob, P)], et_sb[:, kb, :],
                                start=(kb == 0), stop=(kb == nkb - 1),
                            )
                        y_sb = ysb_pool.tile([P, FB], BF16, tag="y_sb")
                        nc.vector.tensor_mul(y_sb[:], ps[:], brecip[:])
                        nc.sync.dma_start(y_ch[b][qb][ts(ob, P), :], y_sb[:])
                    # one 512KB ReduceScatter per chunk: the collectives here
                    # are ring-latency-bound, so fewer+bigger beats per-half
                    nc.gpsimd.collective_compute(
                        "ReduceScatter",
                        mybir.AluOpType.add,
                        replica_groups=[list(range(NC))],
                        ins=[y_ch[b][qb][:]],
                        outs=[rs_all[cb, :, :]],
                    )

            def fin_phase(b):
                # one linear DMA per batch (rs_all's layout == out_ext's);
                # waits on that batch's 8 collectives only
                nb = s // FB
                nc.scalar.dma_start(
                    out_ext[b * nb:(b + 1) * nb, :, :],
                    rs_all[b * nb:(b + 1) * nb, :, :])

            with nc.named_scope("zvw0"):
                zvw_phase(0)
            with nc.named_scope("attn0"):
                attn_phase(0)
            with nc.named_scope("zvw1"):
                zvw_phase(1)
            with nc.named_scope("attn1"):
                attn_phase(1)
            with nc.named_scope("fin0"):
                fin_phase(0)
            with nc.named_scope("fin1"):
                fin_phase(1)

    nc.compile()
    return nc


def _get_nc():
    if "nc" not in _CACHED:
        _CACHED["nc"] = _build()
    return _CACHED["nc"]


def _marshal(x, w_qkv, b_qkv, w_out, b_out):
    x = np.asarray(x, dtype=np.float32)
    w_qkv = np.asarray(w_qkv, dtype=np.float32)
    b_qkv = np.asarray(b_qkv, dtype=np.float32)
    w_out = np.asarray(w_out, dtype=np.float32)

    bf = ml_dtypes.bfloat16
    f8 = ml_dtypes.float8_e4m3
    xt = np.ascontiguousarray(x.reshape(T, D).T)           # [D, T] f32
    xt_bf = xt.astype(bf)
    # c-pair fp8 layout [p, c2, i, t] flattened to [P, 4*T]
    x8 = np.ascontiguousarray(
        xt.reshape(2, 2, P, T).transpose(2, 0, 1, 3).reshape(P, 4 * T)
    ).astype(f8)
    in_maps = []
    for h in range(NC):
        wq = w_qkv[:, h, 0:D]
        wk = w_qkv[:, h, D:2 * D]
        wv = w_qkv[:, h, 2 * D:3 * D]
        wo = w_out[h]
        wqk = np.ascontiguousarray(wq @ wk.T).astype(bf)   # [D, D] (d, d')
        wvo = np.ascontiguousarray(wv @ wo).astype(bf)     # [D, D] (d, o)
        # per-key score bias bq.(x Wk), folded into Exp's bias (pre-scaled)
        ebv = SCALE * (x.reshape(T, D) @ (wk @ b_qkv[h, 0:D]))
        eb = np.ascontiguousarray(ebv.reshape(T // P, P).T.astype(np.float32))
        in_maps.append({
            "xt": xt_bf, "x8": x8, "wqk": wqk, "wvo": wvo, "eb": eb,
        })
    return in_maps


def kernel(x, w_qkv, b_qkv, w_out, b_out):
    x = np.asarray(x)
    b_qkv_np = np.asarray(b_qkv, dtype=np.float32)
    w_out_np = np.asarray(w_out, dtype=np.float32)
    # bv@W_out passes through the softmax-weighted sum as a constant
    b_eff = np.asarray(b_out, dtype=np.float32) + sum(
        b_qkv_np[h, 2 * D:3 * D] @ w_out_np[h] for h in range(NC))
    in_maps = _marshal(x, w_qkv, b_qkv, w_out, b_out)
    nc = _get_nc()
    res = run_bass_kernel_spmd(nc, in_maps, core_ids=list(range(NC)))
    yt = np.empty((D, T), dtype=np.float32)
    for i in range(NC):
        # [chunk, 64, 512]; core i holds output features i*64 + [0, 64)
        o = np.asarray(res.results[i]["out"], dtype=np.float32)
        yt[i * OUT_ROWS:(i + 1) * OUT_ROWS] = o.transpose(1, 0, 2).reshape(
            OUT_ROWS, T)
    yt = yt + b_eff.reshape(D, 1)
    return np.ascontiguousarray(yt.T).reshape(B, S, D).astype(x.dtype)
